# revision 5
# baseline (speedup 1.0000x reference)
"""Multi-head attention (b=4, n=2048, dim=1024, heads=16) on 8 TRN2 cores.

Sharding: tensor-parallel over heads (2 heads per core) + row-parallel output
projection; host sums the 8 partial outputs and adds the bias.

Schedule: scores contract over head_dim=64 only, so the two heads' score
matmuls are issued as row-tiled pairs (tile_position=(0,0)/(64,0)) that run
CONCURRENTLY on disjoint PE row-groups (~2x measured), writing one 4-bank
PSUM supertile [128,4,512] that a single 2048-wide exp drains (one ACT
instruction per head-pair instead of two). The exp'd tiles park in SBUF
until each head's attnV chain consumes them. QKV projection and the output
projection matmuls are interleaved into the slot stream as rationed filler
to keep the PE busy (and p-state ramped) end-to-end.

Per-core math (heads h0=2c, h1=2c+1), one chunk = (batch, query-half, head):
  qkv^T = w_in_c^T @ x^T              (filler, f32 psum, 8-step k chains)
  S^T_pair = K_h^T.T @ Q_h^T          (2x concurrent K=64 row-tiled, bf16)
  E^T   = exp(S^T / 8)                (one ACT over [128,2048], both heads)
  [o^T; denom] = [v_h | 1].T @ E^T    (bf16, M=65 fuses softmax denominator)
  o_norm^T = o^T * (1/denom)          (DVE recip on [128,16] + DMA broadcast)
  partial^T = w_out_c.T @ o_norm^T    (filler) -> DRAM bf16
"""

import os
import sys
import types

import numpy as np

# NTFF-profile hook shim: container's antenv lacks axon_hooks; harmless if
# tracing is never requested.
if "antenv.axon_hooks" not in sys.modules:
    try:
        from trn_agent_boot.trn_boot import _ntff_profile_via_ctypes

        _m = types.ModuleType("antenv.axon_hooks")
        _h = _ntff_profile_via_ctypes("/opt/axon/libaxon_pjrt.so")
        _m.get_axon_ntff_profile_hook = lambda: _h
        _m.set_axon_ntff_profile_hook = lambda hook: None
        sys.modules["antenv.axon_hooks"] = _m
    except Exception:
        pass

import ml_dtypes

import concourse.bacc as bacc
import concourse.bass as bass
import concourse.mybir as mybir
import concourse.tile as tile
from concourse.bass_utils import run_bass_kernel_spmd
from concourse.masks import make_identity

F32 = mybir.dt.float32
BF16 = mybir.dt.bfloat16

B, N, DIM, HEADS = 4, 2048, 1024, 16
HD = DIM // HEADS          # 64
NCORES = 8
HPC = HEADS // NCORES      # 2 heads per core
NT = B * N                 # 8192 tokens
MQKV = 3 * HPC * HD        # 384 qkv output dims per core
SCALE = HD ** -0.5         # 0.125

KT_TILES = DIM // 128      # 8 k-tiles in the projection contraction
JT = N // 128              # 16 j-tiles per batch
NNB = NT // 1024           # 8 token-blocks for qkv
NSLOT = 16 * JT            # 256 slots: (chunk, jt)
WARM = 8                   # superchunk-0 score pairs emitted pre-slot


def _build_nc():
    nc = bacc.Bacc("TRN2", target_bir_lowering=False, debug=False)

    xT = nc.dram_tensor("xT", [DIM, NT], BF16, kind="ExternalInput")
    w_in_c = nc.dram_tensor("w_in_c", [DIM, MQKV], BF16, kind="ExternalInput")
    w_out_c = nc.dram_tensor("w_out_c", [128, DIM], BF16, kind="ExternalInput")
    po = nc.dram_tensor("po", [DIM, NT], BF16, kind="ExternalOutput")
    dn_dram = nc.dram_tensor("dn_dram", [16, 1024], BF16)
    rc_dram = nc.dram_tensor("rc_dram", [16, 1024], BF16)

    xT_r = xT.rearrange("(kt p) n -> p kt n", p=128)

    with tile.TileContext(nc) as tc:
        with (
            tc.tile_pool(name="big", bufs=1) as big,
            tc.tile_pool(name="xinp", bufs=2) as xinp,
            tc.tile_pool(name="strm", bufs=2) as strm,
            tc.tile_pool(name="et", bufs=18) as etp,
            tc.tile_pool(name="pout", bufs=4) as poutp,
            tc.tile_pool(name="stp", bufs=1, space="PSUM") as stp,
            tc.tile_pool(name="fillp", bufs=2, space="PSUM") as fillp,
            tc.tile_pool(name="pop", bufs=1, space="PSUM") as popp,
        ):
            # ---- persistent SBUF ----
            QT = big.tile([128, NT], BF16)    # [q_h0(0:64); q_h1(64:128)]^T
            KT = big.tile([128, NT], BF16)    # [k_h0(0:64); k_h1(64:128)]^T
            # [v_h0 |1| v_h1 |1| zeros]: attnV lhsT h0 = cols 0:128, h1 = 65:193
            Vt = big.tile([128, B * JT, 196], BF16)
            o_sb = big.tile([128, NT], BF16)  # o^T both heads (normed in place)
            w_in_sb = big.tile([128, KT_TILES, MQKV], BF16)
            w_out_sb = big.tile([128, DIM], BF16)

            w_in_r = w_in_c.rearrange("(kt p) m -> p kt m", p=128)
            for m in (1, 0, 2):
                nc.sync.dma_start(
                    out=w_in_sb[:, :, m * 128:(m + 1) * 128],
                    in_=w_in_r[:, :, m * 128:(m + 1) * 128],
                )
            ident = big.tile([128, 128], BF16)
            make_identity(nc, ident)
            # big one-time memsets go to the otherwise-idle gpsimd engine
            nc.gpsimd.memset(Vt[:, :, 129:196], 0.0)
            nc.gpsimd.memset(Vt[:, :, 64], 1.0)
            nc.gpsimd.memset(Vt[:, :, 129], 1.0)

            xin_tiles = {}

            def emit_xin_dma(nb, split=False):
                xin = xinp.tile(
                    [128, KT_TILES, 2, 512], BF16, tag="xin", name=f"xin{nb}"
                )
                ncol = slice(nb * 1024, (nb + 1) * 1024)
                src = xT_r[:, :, ncol].rearrange("p kt (a b) -> p kt a b", b=512)
                if split:
                    # finer slices so the first qkv chains start sooner
                    for a in range(2):
                        for k in range(KT_TILES):
                            nc.sync.dma_start(
                                out=xin[:, k, a, :], in_=src[:, k, a, :]
                            )
                else:
                    for a in range(2):
                        nc.sync.dma_start(
                            out=xin[:, :, a, :], in_=src[:, :, a, :]
                        )
                xin_tiles[nb] = xin

            # ---------- filler unit generators ----------
            def qkv_unit(nb, m, a):
                """8-chain projection matmul unit -> QT/KT/Vt columns."""
                pj = fillp.tile([128, 512], F32, tag="fill", name=f"pj{nb}{m}{a}")
                xin = xin_tiles[nb]
                for k in range(KT_TILES):
                    nc.tensor.matmul(
                        pj,
                        w_in_sb[:, k, m * 128:(m + 1) * 128],
                        xin[:, k, a, :],
                        start=(k == 0),
                        stop=(k == KT_TILES - 1),
                    )
                    yield 1
                cols = slice(nb * 1024 + a * 512, nb * 1024 + (a + 1) * 512)
                if m == 0:
                    nc.vector.tensor_copy(QT[:, cols], pj)
                elif m == 1:
                    nc.vector.tensor_copy(KT[:, cols], pj)
                else:
                    vstage = strm.tile([128, 512], BF16, tag="vstage")
                    nc.vector.tensor_copy(vstage, pj)
                    g0 = nb * 8 + a * 4
                    tp = fillp.tile(
                        [128, 4, 128], BF16, tag="fill", name=f"tp{g0}"
                    )
                    for c in range(4):
                        nc.tensor.transpose(
                            tp[:, c, :], vstage[:, c * 128:(c + 1) * 128], ident
                        )
                        # transpose LDWEIGHTS (~176ns) exceeds its compute
                        # (~92ns): cost 3 ends the pop call so transposes
                        # always have 512-wide matmuls between them
                        yield 3
                    # one strided drain per half frees the psum bank in two
                    # DVE ops instead of eight
                    nc.vector.tensor_copy(Vt[:, g0:g0 + 4, 0:64], tp[:, :, 0:64])
                    nc.vector.tensor_copy(
                        Vt[:, g0:g0 + 4, 65:129], tp[:, :, 64:128]
                    )
                    yield 0

            def proj_unit(bp, ihp, mt, a):
                """One output-projection matmul -> po DRAM (bf16 partial)."""
                i0 = bp * N + ihp * 1024 + a * 512
                pp = fillp.tile(
                    [128, 512], F32, tag="fill", name=f"pp{bp}{ihp}{mt}{a}"
                )
                nc.tensor.matmul(
                    pp,
                    w_out_sb[:, mt * 128:(mt + 1) * 128],
                    o_sb[:, i0:i0 + 512],
                    start=True,
                    stop=True,
                )
                yield 2
                pb = poutp.tile([128, 512], BF16, tag="pout")
                nc.vector.tensor_copy(pb, pp)
                if bp == B - 1 and ihp == 1:
                    # tail: halve the final output DMAs so the post-compute
                    # drain spreads across twice the queues
                    for hq in range(2):
                        nc.sync.dma_start(
                            out=po[mt * 128:(mt + 1) * 128,
                                   i0 + hq * 256:i0 + (hq + 1) * 256],
                            in_=pb[:, hq * 256:(hq + 1) * 256],
                        )
                else:
                    nc.sync.dma_start(
                        out=po[mt * 128:(mt + 1) * 128, i0:i0 + 512], in_=pb
                    )
                yield 0

            def dma_unit(fn, *args):
                fn(*args)
                yield 0

            class Filler:
                def __init__(self):
                    self.q = []
                    self.ndummy = 0
                    self.dummy_tile = None

                def add(self, gen):
                    self.q.append(gen)

                def _dummy(self):
                    if self.dummy_tile is None:
                        self.dummy_tile = fillp.tile(
                            [128, 512], F32, tag="fill",
                            name=f"dj{self.ndummy}",
                        )
                    self.ndummy += 1
                    nc.tensor.matmul(
                        self.dummy_tile, w_out_sb[:, 0:128],
                        w_out_sb[:, 0:512], start=True, stop=True,
                    )

                def dummies(self, n):
                    for _ in range(n):
                        self._dummy()

                def pop(self, n, dummy_ok=True):
                    got = 0
                    while got < n:
                        if not self.q:
                            if not dummy_ok:
                                return got
                            # dummy matmul: keeps the PE p-state ramped when
                            # real filler is exhausted; result never read
                            self._dummy()
                            got += 1
                            continue
                        try:
                            got += next(self.q[0])
                            self.dummy_tile = None
                        except StopIteration:
                            self.q.pop(0)
                    return got

            filler = Filler()

            # ---------- attention machinery ----------
            # chunk ci = (b, ih, h), h fastest; superchunk s = ci // 2
            chunks = [
                (b, ih, h)
                for b in range(B)
                for ih in range(2)
                for h in range(HPC)
            ]
            ets = {}

            def emit_pair(s, jt):
                """Both heads' scores at (superchunk s, key-tile jt):
                4 concurrent row-tiled K=64 matmuls + one 2048-wide exp."""
                b, ih = s // 2, s % 2
                i0 = b * N + ih * 1024
                jcol = slice(b * N + jt * 128, b * N + jt * 128 + 128)
                st = stp.tile([128, 4, 512], F32, tag="st", name=f"st{s}_{jt}")
                for a in range(2):
                    nc.tensor.matmul(
                        st[:, a, :],
                        KT[0:64, jcol],
                        QT[0:64, i0 + a * 512:i0 + (a + 1) * 512],
                        start=True, stop=True,
                        tile_position=(0, 0),
                    )
                    nc.tensor.matmul(
                        st[:, 2 + a, :],
                        KT[64:128, jcol],
                        QT[64:128, i0 + a * 512:i0 + (a + 1) * 512],
                        start=True, stop=True,
                        tile_position=(64, 0),
                    )
                et = etp.tile([128, 2, 1024], BF16, tag="et", name=f"et{s}_{jt}")
                nc.scalar.activation(
                    et.rearrange("p h q -> p (h q)"),
                    st.rearrange("p a b -> p (a b)"),
                    mybir.ActivationFunctionType.Exp,
                    scale=SCALE,
                )
                ets[(s, jt)] = et

            def emit_attnv(b, h, jp, et, po_t):
                for a in range(2):
                    nc.tensor.matmul(
                        po_t[:, a, :],
                        Vt[:, b * JT + jp, h * 65:h * 65 + 128],
                        et[:, h, a * 512:(a + 1) * 512],
                        start=(jp == 0),
                        stop=(jp == JT - 1),
                    )

            def emit_drain(b, ih, h, po_t):
                i0 = b * N + ih * 1024
                icol = slice(i0, i0 + 1024)
                seg = b * 4 + ih * 2 + h
                stage = strm.tile([65, 1024], BF16, tag="stage")
                nc.vector.tensor_copy(
                    stage, po_t.rearrange("p a b -> p (a b)")[0:65, :]
                )
                rows = slice(h * 64, (h + 1) * 64)
                nc.sync.dma_start(out=o_sb[rows, icol], in_=stage[0:64, :])
                nc.sync.dma_start(
                    out=dn_dram[seg:seg + 1, :], in_=stage[64:65, :]
                )

            def emit_normalize(b, ih, hh):
                # per-head normalize, emitted a few slots after the drain so
                # the dn DRAM round trip completes before the DVE touches it
                i0 = b * N + ih * 1024
                icol = slice(i0, i0 + 1024)
                seg = b * 4 + ih * 2 + hh
                dns = strm.tile([128, 8], BF16, tag="dns")
                nc.sync.dma_start(
                    out=dns,
                    in_=dn_dram[seg:seg + 1, :].rearrange(
                        "o (p a) -> (o p) a", p=128
                    ),
                )
                rcc = strm.tile([128, 8], BF16, tag="rcc")
                with nc.allow_low_precision(reason="softmax denom recip"):
                    nc.vector.reciprocal(rcc, dns)
                nc.sync.dma_start(
                    out=rc_dram[seg:seg + 1, :].rearrange(
                        "o (p a) -> (o p) a", p=128
                    ),
                    in_=rcc,
                )
                rows = slice(hh * 64, (hh + 1) * 64)
                bcast = strm.tile([128, 1024], BF16, tag="bcast")
                src = rc_dram[seg:seg + 1, :]
                rbc = bass.AP(
                    tensor=src.tensor,
                    offset=src.offset,
                    ap=[[0, 16]] + list(src.ap)[1:],
                )
                for q in range(4):
                    r16 = slice(hh * 64 + q * 16, hh * 64 + (q + 1) * 16)
                    nc.sync.dma_start(out=bcast[r16, :], in_=rbc)
                nc.vector.tensor_mul(
                    o_sb[rows, icol], o_sb[rows, icol], bcast[rows, :]
                )

            # ---------- prologue ----------
            # xin DMAs: 2 big descriptors per block (issue latency dominates
            # the cold start); nb0 split finer so the first chain starts ~3us
            # after the preamble instead of waiting for the full megabyte
            emit_xin_dma(0, split=True)
            emit_xin_dma(1)
            # k and q units for b0 first: score pairs depend only on them
            for nb in range(2):
                for m in (1, 0):
                    for a in range(2):
                        filler.add(qkv_unit(nb, m, a))
            filler.pop(10 ** 6, dummy_ok=False)

            # w_out + deferred b0 v units + b1..b3 qkv as the filler stream
            nc.sync.dma_start(out=w_out_sb, in_=w_out_c[:, :])
            emit_xin_dma(2)
            emit_xin_dma(3)
            for a in range(2):
                filler.add(qkv_unit(0, 2, a))
            for a in range(2):
                filler.add(qkv_unit(1, 2, a))
            for nb in range(2, NNB):
                if nb + 2 < NNB:
                    filler.add(dma_unit(emit_xin_dma, nb + 2))
                for m in (1, 0, 2):
                    for a in range(2):
                        filler.add(qkv_unit(nb, m, a))

            # warmup: first WARM score pairs of superchunk 0, serialized on
            # the single psum supertile; PE stays busy on qkv filler
            for jt in range(WARM):
                emit_pair(0, jt)
                filler.pop(11)

            # pair emission calendar for the slot loop
            emit_at = [[] for _ in range(NSLOT)]
            for i in range(16 - WARM):
                g = 2 * i if WARM + i < 15 else min(2 * i, 13)
                emit_at[g].append((0, WARM + i))
            for s in range(1, 8):
                for jt in range(JT):
                    emit_at[32 * s - 18 + 2 * jt].append((s, jt))

            # ---------- main slot loop ----------
            pending_norm = []
            pending_proj = []
            po_ts = {}
            acc = 2.0
            for g in range(NSLOT):
                ci, jt = g // JT, g % JT
                b, ih, h = chunks[ci]
                # filler ahead of the pair matmuls: they head-of-line block
                # on the previous supertile's exp, so give the PE real work
                # to chew while the ACT drains
                acc += 1.0
                if int(acc):
                    filler.pop(int(acc))
                    acc -= int(acc)
                for (ps, pjt) in emit_at[g]:
                    emit_pair(ps, pjt)
                if jt == 0:
                    po_ts[ci] = popp.tile(
                        [128, 2, 512], F32, tag="po", name=f"po{ci}"
                    )
                et = ets[(ci // 2, jt)]
                emit_attnv(b, h, jt, et, po_ts[ci])
                if h == 1:
                    del ets[(ci // 2, jt)]
                if jt == JT - 1:
                    emit_drain(b, ih, h, po_ts.pop(ci))
                    pending_norm.append((b, ih, h))
                if jt == 2:
                    while pending_norm:
                        nb_, nih_, nh_ = pending_norm.pop(0)
                        emit_normalize(nb_, nih_, nh_)
                        if nh_ == 1:
                            pending_proj.append((nb_, nih_))
                if jt == 10:
                    # projs join the filler queue well after their chunk's
                    # normalize DMA round trip has retired, so a near-empty
                    # queue never pops a proj that still waits on o_sb
                    while pending_proj:
                        nb_, nih_ = pending_proj.pop(0)
                        for mt in range(DIM // 128):
                            for a in range(2):
                                filler.add(proj_unit(nb_, nih_, mt, a))
                # extra pops at chunk tails cover the drain copy latency
                # before the next chunk's attnV(jt=0) needs the po bank
                rate = 1.1 + (2.0 if jt == JT - 1 else 0.0)
                acc += rate
                npop = int(acc)
                if npop:
                    filler.pop(npop)
                    acc -= npop
                if ci >= 14 and jt >= JT - 2:
                    filler.dummies(3)

            # ---------- epilogue ----------
            # last chunk-pair's normalize + projection; remaining filler
            # covers the dn/rc DMA round trip, then a few dummies hold the
            # p-state until the final proj matmuls retire
            while pending_proj:
                nb_, nih_ = pending_proj.pop(0)
                for mt in range(DIM // 128):
                    for a in range(2):
                        filler.add(proj_unit(nb_, nih_, mt, a))
            filler.pop(12)
            while pending_norm:
                nb_, nih_, nh_ = pending_norm.pop(0)
                emit_normalize(nb_, nih_, nh_)
                if nh_ == 1:
                    filler.dummies(16)
                    for mt in range(DIM // 128):
                        for a in range(2):
                            filler.add(proj_unit(nb_, nih_, mt, a))
            filler.dummies(30)
            filler.pop(10 ** 6, dummy_ok=False)

    nc.finalize()
    return nc


_CACHED = {}


def kernel(x, w_in, w_out, b_out, _trace=False):
    if "nc" not in _CACHED:
        _CACHED["nc"] = _build_nc()
    nc = _CACHED["nc"]

    x2 = np.ascontiguousarray(
        x.reshape(NT, DIM).T.astype(np.float32)
    )  # [DIM, NT]
    in_maps = []
    for c in range(NCORES):
        h0, h1 = HPC * c, HPC * c + 1
        cols = []
        for part in range(3):  # q, k, v
            base = part * DIM
            cols.extend(range(base + h0 * HD, base + h0 * HD + HD))
            cols.extend(range(base + h1 * HD, base + h1 * HD + HD))
        w_in_cc = np.ascontiguousarray(w_in[:, cols].astype(np.float32))
        w_out_cc = np.ascontiguousarray(
            w_out[128 * c:128 * (c + 1), :].astype(np.float32)
        )
        in_maps.append(
            {
                "xT": x2.astype(ml_dtypes.bfloat16),
                "w_in_c": w_in_cc.astype(ml_dtypes.bfloat16),
                "w_out_c": w_out_cc.astype(ml_dtypes.bfloat16),
            }
        )

    res = run_bass_kernel_spmd(
        nc, in_maps, core_ids=list(range(NCORES)), trace=_trace
    )
    acc = res.results[0]["po"].astype(np.float64)
    for c in range(1, NCORES):
        acc = acc + res.results[c]["po"].astype(np.float64)
    out = acc.T + b_out.astype(np.float64)
    if _trace:
        kernel.last_result = res
    return np.ascontiguousarray(out.reshape(B, N, DIM).astype(np.float32))


# revision 6
# speedup vs baseline: 1.1426x; 1.1426x over previous
"""Multi-head attention (b=4, n=2048, dim=1024, heads=16) on 8 TRN2 cores.

Sharding: tensor-parallel over heads (2 heads per core) + row-parallel output
projection; host sums the 8 partial outputs and adds the bias.

Schedule: scores contract over head_dim=64 only, so the two heads' score
matmuls are issued as row-tiled pairs (tile_position=(0,0)/(64,0)) that run
CONCURRENTLY on disjoint PE row-groups (~2x measured), writing one 4-bank
PSUM supertile [128,4,512] that a single 2048-wide exp drains (one ACT
instruction per head-pair instead of two). The exp'd tiles park in SBUF
until each head's attnV chain consumes them. QKV projection and the output
projection matmuls are interleaved into the slot stream as rationed filler
to keep the PE busy (and p-state ramped) end-to-end.

Per-core math (heads h0=2c, h1=2c+1), one chunk = (batch, query-half, head):
  qkv^T = w_in_c^T @ x^T              (filler, f32 psum, 8-step k chains)
  S^T_pair = K_h^T.T @ Q_h^T          (2x concurrent K=64 row-tiled, bf16)
  E^T   = exp(S^T / 8)                (one ACT over [128,2048], both heads)
  [o^T; denom] = [v_h | 1].T @ E^T    (bf16, M=65 fuses softmax denominator)
  o_norm^T = o^T * (1/denom)          (DVE recip on [128,16] + DMA broadcast)
  partial^T = w_out_c.T @ o_norm^T    (filler) -> DRAM bf16
"""

import os
import sys
import types

import numpy as np

# NTFF-profile hook shim: container's antenv lacks axon_hooks; harmless if
# tracing is never requested.
if "antenv.axon_hooks" not in sys.modules:
    try:
        from trn_agent_boot.trn_boot import _ntff_profile_via_ctypes

        _m = types.ModuleType("antenv.axon_hooks")
        _h = _ntff_profile_via_ctypes("/opt/axon/libaxon_pjrt.so")
        _m.get_axon_ntff_profile_hook = lambda: _h
        _m.set_axon_ntff_profile_hook = lambda hook: None
        sys.modules["antenv.axon_hooks"] = _m
    except Exception:
        pass

import ml_dtypes

import concourse.bacc as bacc
import concourse.bass as bass
import concourse.mybir as mybir
import concourse.tile as tile
from concourse.bass_utils import run_bass_kernel_spmd
from concourse.masks import make_identity

F32 = mybir.dt.float32
BF16 = mybir.dt.bfloat16

B, N, DIM, HEADS = 4, 2048, 1024, 16
HD = DIM // HEADS          # 64
NCORES = 8
HPC = HEADS // NCORES      # 2 heads per core
NT = B * N                 # 8192 tokens
MQKV = 3 * HPC * HD        # 384 qkv output dims per core
SCALE = HD ** -0.5         # 0.125

KT_TILES = DIM // 128      # 8 k-tiles in the projection contraction
JT = N // 128              # 16 j-tiles per batch
NNB = NT // 1024           # 8 token-blocks for qkv
NSLOT = 16 * JT            # 256 slots: (chunk, jt)
WARM = 8                   # superchunk-0 score pairs emitted pre-slot


def _build_nc():
    nc = bacc.Bacc("TRN2", target_bir_lowering=False, debug=False)

    xT = nc.dram_tensor("xT", [DIM, NT], BF16, kind="ExternalInput")
    w_in_c = nc.dram_tensor("w_in_c", [DIM, MQKV], BF16, kind="ExternalInput")
    w_out_c = nc.dram_tensor("w_out_c", [128, DIM], BF16, kind="ExternalInput")
    po = nc.dram_tensor("po", [DIM, NT], BF16, kind="ExternalOutput")
    dn_dram = nc.dram_tensor("dn_dram", [16, 1024], BF16)
    rc_dram = nc.dram_tensor("rc_dram", [16, 1024], BF16)

    xT_r = xT.rearrange("(kt p) n -> p kt n", p=128)

    with tile.TileContext(nc) as tc:
        with (
            tc.tile_pool(name="big", bufs=1) as big,
            tc.tile_pool(name="xinp", bufs=2) as xinp,
            tc.tile_pool(name="strm", bufs=2) as strm,
            tc.tile_pool(name="et", bufs=18) as etp,
            tc.tile_pool(name="pout", bufs=4) as poutp,
            tc.tile_pool(name="stp", bufs=1, space="PSUM") as stp,
            tc.tile_pool(name="fillp", bufs=2, space="PSUM") as fillp,
            tc.tile_pool(name="pop", bufs=1, space="PSUM") as popp,
        ):
            # ---- persistent SBUF ----
            QT = big.tile([128, NT], BF16)    # [q_h0(0:64); q_h1(64:128)]^T
            KT = big.tile([128, NT], BF16)    # [k_h0(0:64); k_h1(64:128)]^T
            # [v_h0 |1| v_h1 |1| zeros]: attnV lhsT h0 = cols 0:128, h1 = 65:193
            Vt = big.tile([128, B * JT, 196], BF16)
            o_sb = big.tile([128, NT], BF16)  # o^T both heads (normed in place)
            w_in_sb = big.tile([128, KT_TILES, MQKV], BF16)
            w_out_sb = big.tile([128, DIM], BF16)

            w_in_r = w_in_c.rearrange("(kt p) m -> p kt m", p=128)
            for m in (1, 0, 2):
                nc.sync.dma_start(
                    out=w_in_sb[:, :, m * 128:(m + 1) * 128],
                    in_=w_in_r[:, :, m * 128:(m + 1) * 128],
                )
            ident = big.tile([128, 128], BF16)
            make_identity(nc, ident)
            # big one-time memsets go to the otherwise-idle gpsimd engine
            nc.gpsimd.memset(Vt[:, :, 129:196], 0.0)
            nc.gpsimd.memset(Vt[:, :, 64], 1.0)
            nc.gpsimd.memset(Vt[:, :, 129], 1.0)

            xin_tiles = {}

            def emit_xin_dma(nb, split=False):
                xin = xinp.tile(
                    [128, KT_TILES, 2, 512], BF16, tag="xin", name=f"xin{nb}"
                )
                ncol = slice(nb * 1024, (nb + 1) * 1024)
                src = xT_r[:, :, ncol].rearrange("p kt (a b) -> p kt a b", b=512)
                if split:
                    # finer slices so the first qkv chains start sooner
                    for a in range(2):
                        for k in range(KT_TILES):
                            nc.sync.dma_start(
                                out=xin[:, k, a, :], in_=src[:, k, a, :]
                            )
                else:
                    for a in range(2):
                        nc.sync.dma_start(
                            out=xin[:, :, a, :], in_=src[:, :, a, :]
                        )
                xin_tiles[nb] = xin

            # ---------- filler unit generators ----------
            def qkv_unit(nb, m, a):
                """8-chain projection matmul unit -> QT/KT/Vt columns."""
                pj = fillp.tile([128, 512], F32, tag="fill", name=f"pj{nb}{m}{a}")
                xin = xin_tiles[nb]
                vstage = None
                for k in range(KT_TILES):
                    nc.tensor.matmul(
                        pj,
                        w_in_sb[:, k, m * 128:(m + 1) * 128],
                        xin[:, k, a, :],
                        start=(k == 0),
                        stop=(k == KT_TILES - 1),
                    )
                    if k < KT_TILES - 1:
                        yield 1
                cols = slice(nb * 1024 + a * 512, nb * 1024 + (a + 1) * 512)
                if m == 0:
                    nc.vector.tensor_copy(QT[:, cols], pj)
                    yield 1
                elif m == 1:
                    nc.vector.tensor_copy(KT[:, cols], pj)
                    yield 1
                else:
                    vstage = strm.tile([128, 512], BF16, tag="vstage")
                    nc.vector.tensor_copy(vstage, pj)
                    yield 3
                    g0 = nb * 8 + a * 4
                    tp = fillp.tile(
                        [128, 4, 128], BF16, tag="fill", name=f"tp{g0}"
                    )
                    for c in range(4):
                        nc.tensor.transpose(
                            tp[:, c, :], vstage[:, c * 128:(c + 1) * 128], ident
                        )
                        # transpose LDWEIGHTS (~176ns) exceeds its compute
                        # (~92ns): cost 3 ends the pop call so transposes
                        # always have 512-wide matmuls between them
                        yield 3
                    # one strided drain per half frees the psum bank in two
                    # DVE ops instead of eight
                    nc.vector.tensor_copy(Vt[:, g0:g0 + 4, 0:64], tp[:, :, 0:64])
                    nc.vector.tensor_copy(
                        Vt[:, g0:g0 + 4, 65:129], tp[:, :, 64:128]
                    )
                    yield 0

            def proj_unit(bp, ihp, mt, a):
                """One output-projection matmul -> po DRAM (bf16 partial)."""
                i0 = bp * N + ihp * 1024 + a * 512
                pp = fillp.tile(
                    [128, 512], F32, tag="fill", name=f"pp{bp}{ihp}{mt}{a}"
                )
                nc.tensor.matmul(
                    pp,
                    w_out_sb[:, mt * 128:(mt + 1) * 128],
                    o_sb[:, i0:i0 + 512],
                    start=True,
                    stop=True,
                )
                yield 2
                pb = poutp.tile([128, 512], BF16, tag="pout")
                nc.vector.tensor_copy(pb, pp)
                if bp == B - 1 and ihp == 1:
                    # tail: halve the final output DMAs so the post-compute
                    # drain spreads across twice the queues
                    for hq in range(2):
                        nc.sync.dma_start(
                            out=po[mt * 128:(mt + 1) * 128,
                                   i0 + hq * 256:i0 + (hq + 1) * 256],
                            in_=pb[:, hq * 256:(hq + 1) * 256],
                        )
                else:
                    nc.sync.dma_start(
                        out=po[mt * 128:(mt + 1) * 128, i0:i0 + 512], in_=pb
                    )
                yield 0

            def dma_unit(fn, *args):
                fn(*args)
                yield 0

            class Filler:
                def __init__(self):
                    self.q = []
                    self.ndummy = 0
                    self.dummy_tile = None

                def add(self, gen):
                    self.q.append(gen)

                def _dummy(self):
                    if self.dummy_tile is None:
                        self.dummy_tile = fillp.tile(
                            [128, 512], F32, tag="fill",
                            name=f"dj{self.ndummy}",
                        )
                    self.ndummy += 1
                    nc.tensor.matmul(
                        self.dummy_tile, w_out_sb[:, 0:128],
                        w_out_sb[:, 0:512], start=True, stop=True,
                    )

                def dummies(self, n):
                    for _ in range(n):
                        self._dummy()

                def pop(self, n, dummy_ok=True):
                    got = 0
                    while got < n:
                        if not self.q:
                            if not dummy_ok:
                                return got
                            # dummy matmul: keeps the PE p-state ramped when
                            # real filler is exhausted; result never read
                            self._dummy()
                            got += 1
                            continue
                        try:
                            got += next(self.q[0])
                            self.dummy_tile = None
                        except StopIteration:
                            self.q.pop(0)
                    return got

            filler = Filler()

            # ---------- attention machinery ----------
            # chunk ci = (b, ih, h), h fastest; superchunk s = ci // 2
            chunks = [
                (b, ih, h)
                for b in range(B)
                for ih in range(2)
                for h in range(HPC)
            ]
            ets = {}

            def emit_pair(s, jt):
                """Both heads' scores at (superchunk s, key-tile jt):
                4 concurrent row-tiled K=64 matmuls + one 2048-wide exp."""
                b, ih = s // 2, s % 2
                i0 = b * N + ih * 1024
                jcol = slice(b * N + jt * 128, b * N + jt * 128 + 128)
                st = stp.tile([128, 4, 512], F32, tag="st", name=f"st{s}_{jt}")
                for a in range(2):
                    nc.tensor.matmul(
                        st[:, a, :],
                        KT[0:64, jcol],
                        QT[0:64, i0 + a * 512:i0 + (a + 1) * 512],
                        start=True, stop=True,
                        tile_position=(0, 0),
                    )
                    nc.tensor.matmul(
                        st[:, 2 + a, :],
                        KT[64:128, jcol],
                        QT[64:128, i0 + a * 512:i0 + (a + 1) * 512],
                        start=True, stop=True,
                        tile_position=(64, 0),
                    )
                et = etp.tile([128, 2, 1024], BF16, tag="et", name=f"et{s}_{jt}")
                nc.scalar.activation(
                    et.rearrange("p h q -> p (h q)"),
                    st.rearrange("p a b -> p (a b)"),
                    mybir.ActivationFunctionType.Exp,
                    scale=SCALE,
                )
                ets[(s, jt)] = et

            def emit_attnv(b, h, jp, et, po_t):
                for a in range(2):
                    nc.tensor.matmul(
                        po_t[:, a, :],
                        Vt[:, b * JT + jp, h * 65:h * 65 + 128],
                        et[:, h, a * 512:(a + 1) * 512],
                        start=(jp == 0),
                        stop=(jp == JT - 1),
                    )

            def emit_drain(b, ih, h, po_t):
                i0 = b * N + ih * 1024
                icol = slice(i0, i0 + 1024)
                seg = b * 4 + ih * 2 + h
                stage = strm.tile([65, 1024], BF16, tag="stage")
                nc.vector.tensor_copy(
                    stage, po_t.rearrange("p a b -> p (a b)")[0:65, :]
                )
                rows = slice(h * 64, (h + 1) * 64)
                nc.sync.dma_start(out=o_sb[rows, icol], in_=stage[0:64, :])
                nc.sync.dma_start(
                    out=dn_dram[seg:seg + 1, :], in_=stage[64:65, :]
                )

            def emit_normalize(b, ih, hh):
                # per-head normalize, emitted a few slots after the drain so
                # the dn DRAM round trip completes before the DVE touches it
                i0 = b * N + ih * 1024
                icol = slice(i0, i0 + 1024)
                seg = b * 4 + ih * 2 + hh
                dns = strm.tile([128, 8], BF16, tag="dns")
                nc.sync.dma_start(
                    out=dns,
                    in_=dn_dram[seg:seg + 1, :].rearrange(
                        "o (p a) -> (o p) a", p=128
                    ),
                )
                rcc = strm.tile([128, 8], BF16, tag="rcc")
                with nc.allow_low_precision(reason="softmax denom recip"):
                    nc.vector.reciprocal(rcc, dns)
                nc.sync.dma_start(
                    out=rc_dram[seg:seg + 1, :].rearrange(
                        "o (p a) -> (o p) a", p=128
                    ),
                    in_=rcc,
                )
                rows = slice(hh * 64, (hh + 1) * 64)
                bcast = strm.tile([128, 1024], BF16, tag="bcast")
                src = rc_dram[seg:seg + 1, :]
                rbc = bass.AP(
                    tensor=src.tensor,
                    offset=src.offset,
                    ap=[[0, 16]] + list(src.ap)[1:],
                )
                for q in range(4):
                    r16 = slice(hh * 64 + q * 16, hh * 64 + (q + 1) * 16)
                    nc.sync.dma_start(out=bcast[r16, :], in_=rbc)
                nc.vector.tensor_mul(
                    o_sb[rows, icol], o_sb[rows, icol], bcast[rows, :]
                )

            # ---------- prologue ----------
            # xin DMAs: 2 big descriptors per block (issue latency dominates
            # the cold start); nb0 split finer so the first chain starts ~3us
            # after the preamble instead of waiting for the full megabyte
            emit_xin_dma(0, split=True)
            emit_xin_dma(1)
            # k and q units for b0 first: score pairs depend only on them
            for nb in range(2):
                for m in (1, 0):
                    for a in range(2):
                        filler.add(qkv_unit(nb, m, a))
            filler.pop(10 ** 6, dummy_ok=False)

            # w_out + deferred b0 v units + b1..b3 qkv as the filler stream
            nc.sync.dma_start(out=w_out_sb, in_=w_out_c[:, :])
            emit_xin_dma(2)
            emit_xin_dma(3)
            for a in range(2):
                filler.add(qkv_unit(0, 2, a))
            for a in range(2):
                filler.add(qkv_unit(1, 2, a))
            for nb in range(2, NNB):
                if nb + 2 < NNB:
                    filler.add(dma_unit(emit_xin_dma, nb + 2))
                for m in (1, 0, 2):
                    for a in range(2):
                        filler.add(qkv_unit(nb, m, a))

            # warmup: first WARM score pairs of superchunk 0, serialized on
            # the single psum supertile; PE stays busy on qkv filler
            for jt in range(WARM):
                emit_pair(0, jt)
                filler.pop(11)

            # pair emission calendar for the slot loop
            emit_at = [[] for _ in range(NSLOT)]
            for i in range(16 - WARM):
                g = 2 * i if WARM + i < 15 else min(2 * i, 13)
                emit_at[g].append((0, WARM + i))
            for s in range(1, 8):
                for jt in range(JT):
                    emit_at[32 * s - 18 + 2 * jt].append((s, jt))

            # ---------- main slot loop ----------
            pending_norm = []
            pending_proj = []
            po_ts = {}
            acc = 2.0
            for g in range(NSLOT):
                ci, jt = g // JT, g % JT
                b, ih, h = chunks[ci]
                # filler ahead of the pair matmuls: they head-of-line block
                # on the previous supertile's exp, so give the PE real work
                # to chew while the ACT drains
                acc += 1.0
                if int(acc):
                    filler.pop(int(acc), dummy_ok=False)
                    acc -= int(acc)
                for (ps, pjt) in emit_at[g]:
                    emit_pair(ps, pjt)
                if jt == 0:
                    po_ts[ci] = popp.tile(
                        [128, 2, 512], F32, tag="po", name=f"po{ci}"
                    )
                et = ets[(ci // 2, jt)]
                emit_attnv(b, h, jt, et, po_ts[ci])
                if h == 1:
                    del ets[(ci // 2, jt)]
                if jt == JT - 1:
                    emit_drain(b, ih, h, po_ts.pop(ci))
                    pending_norm.append((b, ih, h))
                if jt == 2:
                    while pending_norm:
                        nb_, nih_, nh_ = pending_norm.pop(0)
                        emit_normalize(nb_, nih_, nh_)
                        if nh_ == 1:
                            pending_proj.append((nb_, nih_))
                if jt == 10:
                    # projs join the filler queue well after their chunk's
                    # normalize DMA round trip has retired, so a near-empty
                    # queue never pops a proj that still waits on o_sb
                    while pending_proj:
                        nb_, nih_ = pending_proj.pop(0)
                        for mt in range(DIM // 128):
                            for a in range(2):
                                filler.add(proj_unit(nb_, nih_, mt, a))
                # extra pops at chunk tails cover the drain copy latency
                # before the next chunk's attnV(jt=0) needs the po bank
                rate = 1.1 + (2.0 if jt == JT - 1 else 0.0)
                acc += rate
                npop = int(acc)
                if npop:
                    filler.pop(npop, dummy_ok=False)
                    acc -= npop
                if ci >= 14 and jt >= JT - 2:
                    filler.dummies(3)

            # ---------- epilogue ----------
            # last chunk-pair's normalize + projection; remaining filler
            # covers the dn/rc DMA round trip, then a few dummies hold the
            # p-state until the final proj matmuls retire
            while pending_proj:
                nb_, nih_ = pending_proj.pop(0)
                for mt in range(DIM // 128):
                    for a in range(2):
                        filler.add(proj_unit(nb_, nih_, mt, a))
            filler.pop(12)
            while pending_norm:
                nb_, nih_, nh_ = pending_norm.pop(0)
                emit_normalize(nb_, nih_, nh_)
                if nh_ == 1:
                    filler.dummies(16)
                    for mt in range(DIM // 128):
                        for a in range(2):
                            filler.add(proj_unit(nb_, nih_, mt, a))
            filler.dummies(30)
            filler.pop(10 ** 6, dummy_ok=False)

    nc.finalize()
    return nc


_CACHED = {}


def kernel(x, w_in, w_out, b_out, _trace=False):
    if "nc" not in _CACHED:
        _CACHED["nc"] = _build_nc()
    nc = _CACHED["nc"]

    x2 = np.ascontiguousarray(
        x.reshape(NT, DIM).T.astype(np.float32)
    )  # [DIM, NT]
    in_maps = []
    for c in range(NCORES):
        h0, h1 = HPC * c, HPC * c + 1
        cols = []
        for part in range(3):  # q, k, v
            base = part * DIM
            cols.extend(range(base + h0 * HD, base + h0 * HD + HD))
            cols.extend(range(base + h1 * HD, base + h1 * HD + HD))
        w_in_cc = np.ascontiguousarray(w_in[:, cols].astype(np.float32))
        w_out_cc = np.ascontiguousarray(
            w_out[128 * c:128 * (c + 1), :].astype(np.float32)
        )
        in_maps.append(
            {
                "xT": x2.astype(ml_dtypes.bfloat16),
                "w_in_c": w_in_cc.astype(ml_dtypes.bfloat16),
                "w_out_c": w_out_cc.astype(ml_dtypes.bfloat16),
            }
        )

    res = run_bass_kernel_spmd(
        nc, in_maps, core_ids=list(range(NCORES)), trace=_trace
    )
    acc = res.results[0]["po"].astype(np.float64)
    for c in range(1, NCORES):
        acc = acc + res.results[c]["po"].astype(np.float64)
    out = acc.T + b_out.astype(np.float64)
    if _trace:
        kernel.last_result = res
    return np.ascontiguousarray(out.reshape(B, N, DIM).astype(np.float32))
